# revision 5
# baseline (speedup 1.0000x reference)
"""DenseEnergyLoss on 8 Trainium2 NeuronCores (Bass/Tile).

Reference computes, per image: a [P,P] Gaussian bilateral affinity
Wm = exp(-0.5*d2(f_p,f_q)) over 5-dim features f = (x/sxy, y/sxy, rgb/15),
then loss = -W/N * sum(S * ((S @ Wm) * gate)) with S = seg_roi, P = 64*64.

Device formulation (per core = one image half, data-parallel over N*2):
  exponent X[p,q] = u_p . v_q with u = [f, -0.5|f|^2, 1], v = [f, 1, -0.5|f|^2]
  (so X = -0.5*d2 exactly), computed as a K=21 fp16 matmul using an
  error-compensated hi/lo split stacked along the contraction dim:
     U = [u_hi, u_hi, u_lo], V = [v_hi, v_lo, v_hi]  ->  X ~ u.v to ~1e-4 abs.
  Wm = exp(X) on the scalar engine (PSUM->SBUF, fp16 out; fp16 rounding
  absorbs the reference's d2>=0 clamp since X <= ~1e-4 when d2==0).
  AS[k,q] += ST_i^T @ Wm accumulated over p-blocks in PSUM (fp32), then
  per q-chunk: (AS * SG) product + free-dim reduce on the vector engine.

Host does only the cheap O(P) prep: stride-2 subsample (nearest resize),
2x2 avg pool (bilinear resize at scale 0.5), gating, feature build, and the
final sum of the 8 per-core partials.
"""

import numpy as np

# problem shapes (hardcoded per contract)
N_IMG = 4
K = 21
H = 128
W = 128
HO, WO = 64, 64
P = HO * WO            # 4096
HALF = P // 2          # p-rows per core
NBLK = HALF // 128     # 16 p-blocks of 128 per core
QCH = 1024             # q-chunk width (2 PSUM banks)
NCH = P // QCH
N_CORES = 8
KF = 21                # contraction dim of the feature matmul (3*7)

SIGMA_RGB = 15.0
SXY = 100.0 * 0.5      # SIGMA_XY * SCALE
WEIGHT = 1e-7

_CACHE = {}


def _build_module(loop_n=1):
    from contextlib import ExitStack

    import concourse.bacc as bacc
    import concourse.tile as tile
    from concourse import mybir

    fp32 = mybir.dt.float32
    fp16 = mybir.dt.float16

    nc = bacc.Bacc(trn_type="TRN2", target_bir_lowering=False, debug=False)

    UH = nc.declare_dram_parameter("UH", [KF, HALF], fp16, isOutput=False)
    VH = nc.declare_dram_parameter("VH", [KF, P], fp16, isOutput=False)
    ST = nc.declare_dram_parameter("ST", [128, NBLK * K], fp16, isOutput=False)
    SG = nc.declare_dram_parameter("SG", [K, P], fp32, isOutput=False)
    OUT = nc.declare_dram_parameter("out", [K, 1], fp32, isOutput=True)

    with tile.TileContext(nc) as tc, ExitStack() as ctx:
        singles = ctx.enter_context(tc.tile_pool(name="singles", bufs=1))
        gpool = ctx.enter_context(tc.tile_pool(name="g", bufs=2, space="PSUM"))
        aspool = ctx.enter_context(tc.tile_pool(name="as", bufs=2, space="PSUM"))
        wpool = ctx.enter_context(tc.tile_pool(name="wm", bufs=3))
        epool = ctx.enter_context(tc.tile_pool(name="evac", bufs=2))
        accp = ctx.enter_context(tc.tile_pool(name="acc", bufs=2))

        sb_UH = singles.tile([KF, HALF], fp16)
        nc.sync.dma_start(out=sb_UH, in_=UH.ap())
        sb_VH = singles.tile([KF, P], fp16)
        nc.sync.dma_start(out=sb_VH, in_=VH.ap())
        sb_ST = singles.tile([128, NBLK * K], fp16)
        nc.sync.dma_start(out=sb_ST, in_=ST.ap())
        sb_SG = singles.tile([K, P], fp32)
        nc.sync.dma_start(out=sb_SG, in_=SG.ap())

        def body():
            cols = accp.tile([K, NCH], fp32)
            for c in range(NCH):
                AS = aspool.tile([K, QCH], fp32)
                for i in range(NBLK):
                    G = gpool.tile([128, QCH], fp32)
                    Wm = wpool.tile([128, QCH], fp16)
                    for h in range(2):
                        nc.tensor.matmul(
                            G[:, h * 512:(h + 1) * 512],
                            lhsT=sb_UH[:, i * 128:(i + 1) * 128],
                            rhs=sb_VH[:, c * QCH + h * 512: c * QCH + (h + 1) * 512],
                            start=True,
                            stop=True,
                            skip_group_check=True,
                        )
                    nc.scalar.activation(
                        out=Wm, in_=G, func=mybir.ActivationFunctionType.Exp
                    )
                    for h in range(2):
                        nc.tensor.matmul(
                            AS[:, h * 512:(h + 1) * 512],
                            lhsT=sb_ST[:, i * K:(i + 1) * K],
                            rhs=Wm[:, h * 512:(h + 1) * 512],
                            start=(i == 0),
                            stop=(i == NBLK - 1),
                            skip_group_check=True,
                        )
                prod = epool.tile([K, QCH], fp32)
                nc.vector.tensor_tensor(
                    out=prod,
                    in0=AS,
                    in1=sb_SG[:, c * QCH:(c + 1) * QCH],
                    op=mybir.AluOpType.mult,
                )
                nc.vector.reduce_sum(
                    out=cols[:, c:c + 1], in_=prod, axis=mybir.AxisListType.X
                )
            acc = accp.tile([K, 1], fp32)
            nc.vector.reduce_sum(out=acc, in_=cols, axis=mybir.AxisListType.X)
            return acc

        if loop_n == 1:
            acc = body()
        else:
            with tc.For_i(0, loop_n) as _:
                acc = body()
        nc.sync.dma_start(out=OUT.ap(), in_=acc)

    nc.compile()
    return nc


def get_module(loop_n=1):
    key = ("nc", loop_n)
    if key not in _CACHE:
        _CACHE[key] = _build_module(loop_n)
    return _CACHE[key]


def preprocess(images, segmentations, ROIs, seg_label):
    """Host-side prep: resizes, gating, feature build, per-core sharding."""
    images = np.asarray(images, dtype=np.float32)
    seg = np.asarray(segmentations, dtype=np.float32)
    roi = np.asarray(ROIs, dtype=np.float32)
    lbl = np.asarray(seg_label)

    img_s = images[:, :, ::2, ::2]                    # nearest resize x0.5
    roi_s = roi[:, ::2, ::2]
    lbl_s = lbl[:, :, ::2, ::2]
    seg_s = 0.25 * (seg[:, :, ::2, ::2] + seg[:, :, 1::2, ::2]
                    + seg[:, :, ::2, 1::2] + seg[:, :, 1::2, 1::2])

    unlabel = (lbl_s == 255)[:, 0]
    gate = np.maximum(
        np.where(unlabel, np.float32(1.0), roi_s - seg_s.max(axis=1)), 0.0
    ).astype(np.float32)
    S = (seg_s * roi_s[:, None]).reshape(N_IMG, K, P).astype(np.float32)
    SG = (S * gate.reshape(N_IMG, 1, P)).astype(np.float32)

    yy, xx = np.meshgrid(np.arange(HO, dtype=np.float32),
                         np.arange(WO, dtype=np.float32), indexing="ij")
    pos = np.stack([xx.ravel() / SXY, yy.ravel() / SXY], axis=-1)  # [P,2]

    in_maps = []
    for n in range(N_IMG):
        col = img_s[n].reshape(3, P).T / SIGMA_RGB
        f = np.concatenate([pos, col], axis=-1).astype(np.float32)  # [P,5]
        sq = np.sum(f * f, axis=-1)
        ones = np.ones((P, 1), np.float32)
        u = np.concatenate([f, -0.5 * sq[:, None], ones], axis=1)   # [P,7]
        v = np.concatenate([f, ones, -0.5 * sq[:, None]], axis=1)
        uh = u.astype(np.float16)
        ul = (u - uh.astype(np.float32)).astype(np.float16)
        vh = v.astype(np.float16)
        vl = (v - vh.astype(np.float32)).astype(np.float16)
        U = np.concatenate([uh, uh, ul], axis=1)                    # [P,21] fp16
        V = np.concatenate([vh, vl, vh], axis=1)                    # [P,21] fp16
        ST_full = S[n].T.astype(np.float16)                         # [P,K]
        for hh in range(2):
            sl = slice(hh * HALF, (hh + 1) * HALF)
            st = (ST_full[sl].reshape(NBLK, 128, K)
                  .transpose(1, 0, 2).reshape(128, NBLK * K))
            in_maps.append({
                "UH": np.ascontiguousarray(U[sl].T),   # [21, HALF] fp16
                "VH": np.ascontiguousarray(V.T),       # [21, P] fp16
                "ST": np.ascontiguousarray(st),        # [128, NBLK*K] fp16
                "SG": SG[n],                           # [K, P] fp32
            })
    return in_maps


def kernel(images, segmentations, ROIs, seg_label):
    from concourse.bass_utils import run_bass_kernel_spmd

    nc = get_module()
    in_maps = preprocess(images, segmentations, ROIs, seg_label)
    res = run_bass_kernel_spmd(nc, in_maps, list(range(N_CORES)))
    total = 0.0
    for r in res.results:
        total += float(r["out"].sum())
    return np.array([-WEIGHT * total / N_IMG], dtype=np.float32)
